# revision 1
# baseline (speedup 1.0000x reference)
# GAT layer kernel for Trainium2 (Bass/Tile), data-parallel over batch:
# one graph per NeuronCore, 8 cores.
#
# Math (per graph, N=2048 nodes, F=128 in, O=64 out):
#   Wh = h @ W + b
#   e[i,j] = leakyrelu(s1[i] + s2[j], 0.2),  s1 = Wh@a1, s2 = Wh@a2
#   att = softmax over i of where(adj>0, e, -inf)
#   out = elu(att^T @ Wh)
#
# Kernel formulation:
#   s1 = h@(W a1) + b.a1, s2 = h@(W a2) + b.a2  (skip materializing Wh^T)
#   v = exp(0.2*(s1[i] + s2[j])); exp(leakyrelu(t)) = max(v^5, v) since
#   v^5 = exp(t) and exp is monotone. One custom DVE op computes
#   P = select(adj, max(v^5, v), 0) in a single 1x pass.
#   Softmax denominator: ones-column appended to Wh; accT = [Wh|1]^T @ P
#   holds the per-column sums in row O. No max-subtraction needed (logits
#   bounded for this data; masked entries are exactly 0).
#   elu(x) = max(x, exp(min(x, 0)) - 1).

import numpy as np

import concourse.bacc as bacc
import concourse.mybir as mybir
import concourse.tile as tile
from concourse import masks
from concourse import dve_ops as dvo
from concourse.dve_spec import (
    Spec, Src0, Src1, Zero, maxx, select,
    _has_src1 as has_src1, lower as dve_lower,
)
from concourse.dve_uop import DveOpSpec
from concourse.bass_utils import run_bass_kernel_spmd
from contextlib import ExitStack


def _register_gat_op():
    """Custom DVE op: P = select(adj != 0, max(v^5, v), 0)."""
    name = "GAT_EXP5_MASK_ANT"
    for o in dvo.OPS:
        if o.name == name:
            return o
    v2 = Src0 * Src0
    v4 = v2 * v2
    body = select(Src1, maxx(v4 * Src0, Src0), Zero)

    def _ref(in0, in1, s0, s1, imm2):
        f = in0.astype(np.float32)
        u = ((f * f) * (f * f)) * f
        return np.where(in1 != 0, np.maximum(u, f), np.float32(0.0)).astype(
            np.float32)

    spec = Spec(body=body, reference=_ref)
    row = dvo._CUSTOM_DVE_ROW_BASE + len(dvo.OPS)
    assert row < 0x20, "custom DVE opcode rows exhausted"
    shas = {}
    for ver in ("v3", "v4"):
        tmp = DveOpSpec(name=name, opcode=row, uops=dve_lower(spec, ver=ver),
                        rd1_en=has_src1(spec))
        shas[ver] = tmp.sha(ver)
    op = dvo.DveOp(name, spec, subdim=False, uops_sha=shas)
    dvo.OPS.append(op)
    dvo._SUB_OPCODE_FOR_NAME[name] = row
    return op


GAT_OP = _register_gat_op()


def _register_gat_op2():
    """Residual op: Pl = select(ph != 0, max(v^5, v) - ph, 0).

    ph is the tf32-rounded masked P from GAT_EXP5_MASK_ANT; ph > 0 exactly
    where adj != 0 (exp values are strictly positive and tf32 keeps the
    full fp32 exponent range), so ph doubles as the mask. Writing the
    output as float32r yields the second tf32 limb of P: Ph + Pl carries
    ~22 mantissa bits, recovering fp32-grade precision with f32r matmuls.
    """
    name = "GAT_EXP5_RES_ANT"
    for o in dvo.OPS:
        if o.name == name:
            return o
    v2 = Src0 * Src0
    v4 = v2 * v2
    body = select(Src1, maxx(v4 * Src0, Src0) - Src1, Zero)

    def _ref(in0, in1, s0, s1, imm2):
        f = in0.astype(np.float32)
        u = ((f * f) * (f * f)) * f
        return np.where(in1 != 0, np.maximum(u, f) - in1,
                        np.float32(0.0)).astype(np.float32)

    spec = Spec(body=body, reference=_ref)
    row = dvo._CUSTOM_DVE_ROW_BASE + len(dvo.OPS)
    assert row < 0x20, "custom DVE opcode rows exhausted"
    shas = {}
    for ver in ("v3", "v4"):
        tmp = DveOpSpec(name=name, opcode=row, uops=dve_lower(spec, ver=ver),
                        rd1_en=has_src1(spec))
        shas[ver] = tmp.sha(ver)
    op = dvo.DveOp(name, spec, subdim=False, uops_sha=shas)
    dvo.OPS.append(op)
    dvo._SUB_OPCODE_FOR_NAME[name] = row
    return op


GAT_OP2 = _register_gat_op2()

N = 2048
F = 128
O = 64
B = 8
ALPHA = 0.2

F32 = mybir.dt.float32
F32R = mybir.dt.float32r
BF16 = mybir.dt.bfloat16
I32 = mybir.dt.int32
AF = mybir.ActivationFunctionType
ALU = mybir.AluOpType

RT = N // 128   # 16 row blocks of 128
CW = 512        # matmul chunk width (one PSUM bank of fp32)
CT = N // CW    # 4 chunks
ET = N // 128   # 16 epilogue chunks
RB = 2          # row blocks merged per adjacency DMA


def build_gat_kernel(repeat=1, fused=True, hw_loop=False, mm_f32r=False,
                     dma_only=False, adj_bufs=4, stage1_once=False, wh_comp=False,
                     uv_bufs=3, p_comp=False):
    nc = bacc.Bacc("TRN2", target_bir_lowering=False, debug=False, num_devices=B)

    h = nc.dram_tensor("h", [N, F], F32, kind="ExternalInput").ap()
    adj = nc.dram_tensor("adj", [N, N], I32, kind="ExternalInput").ap()
    W = nc.dram_tensor("W", [F, O], F32, kind="ExternalInput").ap()
    bvec = nc.dram_tensor("b", [O], F32, kind="ExternalInput").ap()
    avec = nc.dram_tensor("a", [2 * O, 1], F32, kind="ExternalInput").ap()
    y = nc.dram_tensor("y", [N, O], F32, kind="ExternalOutput").ap()

    mmdt = F32R if mm_f32r else F32

    with tile.TileContext(nc) as tc, ExitStack() as ctx:
        const = ctx.enter_context(tc.tile_pool(name="const", bufs=1))
        ld = ctx.enter_context(tc.tile_pool(name="ld", bufs=2))
        ps = ctx.enter_context(tc.tile_pool(name="ps", bufs=2, space="PSUM"))
        ps_acc = ctx.enter_context(tc.tile_pool(name="ps_acc", bufs=1, space="PSUM"))
        ps_ep = ctx.enter_context(tc.tile_pool(name="ps_ep", bufs=2, space="PSUM"))
        adj_pool = ctx.enter_context(tc.tile_pool(name="adjp", bufs=adj_bufs))
        uv_pool = ctx.enter_context(tc.tile_pool(name="uvp", bufs=uv_bufs))
        p_pool = ctx.enter_context(tc.tile_pool(name="pp", bufs=3))
        ep_pool = ctx.enter_context(tc.tile_pool(name="epp", bufs=4))

        # ---------- one-time constants ----------
        ident = const.tile([128, 128], F32)
        masks.make_identity(nc, ident[:])

        W_sb = const.tile([F, O], F32)
        nc.sync.dma_start(W_sb[:], W)
        a1_sb = const.tile([O, 1], F32)
        nc.sync.dma_start(a1_sb[:], avec[:O, :])
        a2_sb = const.tile([O, 1], F32)
        nc.sync.dma_start(a2_sb[:], avec[O:, :])
        b_sb = const.tile([1, O], F32)
        nc.sync.dma_start(b_sb[:], bvec[None, :])
        bcol_sb = const.tile([O, 1], F32)
        nc.sync.dma_start(bcol_sb[:], bvec[:, None])
        ones_row = const.tile([1, 128], F32)
        nc.vector.memset(ones_row[:], 1.0)

        # [W | 0] and broadcast [b | 1] so whm = hT@[W|0] + [b|1] lands the
        # ones column (softmax denominator) without a per-block memset.
        W1_sb = const.tile([F, O + 1], F32)
        nc.vector.memset(W1_sb[:, O:O + 1], 0.0)
        nc.sync.dma_start(W1_sb[:, :O], W)
        b1row = const.tile([1, O + 1], F32)
        nc.vector.memset(b1row[:, O:O + 1], 1.0)
        nc.sync.dma_start(b1row[:, :O], bvec[None, :])
        pb = ps.tile([128, O + 1], F32, tag="s1ps")
        nc.tensor.matmul(pb[:], ones_row[:], b1row[:], start=True, stop=True)
        b_bc1 = const.tile([128, O + 1], F32)
        nc.scalar.copy(b_bc1[:], pb[:])

        # W^T, then w1 = W@a1, w2 = W@a2 as [F,1] columns
        pwt = ps.tile([O, 128], F32, tag="s1ps")
        nc.tensor.transpose(pwt[:], W_sb[:], ident[:])
        wT = const.tile([O, 128], F32)
        nc.scalar.copy(wT[:], pwt[:])
        pw1 = ps.tile([128, 1], F32, tag="s1ps")
        nc.tensor.matmul(pw1[:], wT[:], a1_sb[:], start=True, stop=True)
        w1 = const.tile([128, 1], F32)
        nc.scalar.copy(w1[:], pw1[:])
        pw2 = ps.tile([128, 1], F32, tag="s1ps")
        nc.tensor.matmul(pw2[:], wT[:], a2_sb[:], start=True, stop=True)
        w2 = const.tile([128, 1], F32)
        nc.scalar.copy(w2[:], pw2[:])

        # beta1 = b.a1, beta2 = b.a2, broadcast to [128,1]
        pb1 = ps.tile([1, 1], F32, tag="s1ps")
        nc.tensor.matmul(pb1[:], bcol_sb[:], a1_sb[:], start=True, stop=True)
        b1_sb = const.tile([1, 1], F32)
        nc.scalar.copy(b1_sb[:], pb1[:])
        pb2 = ps.tile([1, 1], F32, tag="s1ps")
        nc.tensor.matmul(pb2[:], bcol_sb[:], a2_sb[:], start=True, stop=True)
        b2_sb = const.tile([1, 1], F32)
        nc.scalar.copy(b2_sb[:], pb2[:])
        pb1b = ps.tile([128, 1], F32, tag="s1ps")
        nc.tensor.matmul(pb1b[:], ones_row[:], b1_sb[:], start=True, stop=True)
        b1_bc = const.tile([128, 1], F32)
        nc.scalar.copy(b1_bc[:], pb1b[:])
        b1f_bc = const.tile([128, 1], F32)
        nc.scalar.mul(b1f_bc[:], pb1b[:], ALPHA)
        pb2b = ps.tile([128, 1], F32, tag="s1ps")
        nc.tensor.matmul(pb2b[:], ones_row[:], b2_sb[:], start=True, stop=True)
        b2_bc = const.tile([128, 1], F32)
        nc.scalar.copy(b2_bc[:], pb2b[:])

        # Warm the Exp activation-table set up front so the ~2.7us table
        # load overlaps the first DMAs instead of stalling the first v-exp.
        warm = const.tile([1, 1], F32)
        nc.scalar.activation(warm[:], ones_row[:, :1], AF.Exp)

        # ---------- per-iteration persistent tiles ----------
        hT = const.tile([128, N], F32)      # [f, n]
        s1_sb = const.tile([128, RT], F32)  # s1 per row-block (unfused path)
        s1f_sb = const.tile([128, RT], F32)  # 0.2*s1 + 0.2*beta1
        s2_row = const.tile([1, N], F32)
        s2_bc = const.tile([128, N], F32)
        accT = const.tile([O + 1, N], F32)
        out_sb = const.tile([128, ET, O], F32)

        wh_mm_shared = []

        def _stage1():
            hload = ld.tile([128, RT, F], F32, tag="hload")
            hsrc = h.rearrange("(r p) f -> p r f", p=128)
            HH = RT // 2
            nc.sync.dma_start(hload[:, :HH, :], hsrc[:, :HH, :])
            nc.sync.dma_start(hload[:, HH:, :], hsrc[:, HH:, :])
            rng = range(RT) if not dma_only else range(0)
            for r in rng:
                rsl = slice(r * 128, (r + 1) * 128)
                pt = ps.tile([128, 128], F32, tag="s1ps")
                nc.tensor.transpose(pt[:], hload[:, r, :], ident[:])
                nc.scalar.copy(hT[:, rsl], pt[:])

            # s2 row then broadcast to all partitions (with beta2 bias)
            for c in range(CT) if not dma_only else range(0):
                csl = slice(c * CW, (c + 1) * CW)
                ps2 = ps.tile([1, CW], F32, tag="s1ps")
                nc.tensor.matmul(ps2[:], w2[:], hT[:, csl], start=True,
                                 stop=True)
                nc.scalar.copy(s2_row[:, csl], ps2[:])
                pbc = ps.tile([128, CW], F32, tag="s1ps")
                nc.tensor.matmul(pbc[:], ones_row[:], s2_row[:, csl],
                                 start=True, stop=True)
                nc.scalar.activation(s2_bc[:, csl], pbc[:], AF.Identity,
                                     bias=b2_bc[:])

            wh_mm_tiles = []
            for r in rng:
                rsl = slice(r * 128, (r + 1) * 128)
                # s1 terms for this block
                ps1 = ps.tile([128, 1], F32, tag="s1ps")
                nc.tensor.matmul(ps1[:], hT[:, rsl], w1[:], start=True,
                                 stop=True)
                nc.vector.scalar_tensor_tensor(
                    s1f_sb[:, r:r + 1], ps1[:], ALPHA, b1f_bc[:],
                    op0=ALU.mult, op1=ALU.add)
                if not fused:
                    nc.vector.tensor_tensor(s1_sb[:, r:r + 1], ps1[:],
                                            b1_bc[:], op=ALU.add)

                # Wh (+ ones column) for the big matmul, in matmul dtype
                pw = ps.tile([128, O + 1], F32, tag="s1ps")
                nc.tensor.matmul(pw[:], hT[:, rsl], W1_sb[:], start=True,
                                 stop=True)
                if wh_comp:
                    whf = const.tile([128, O + 1], F32, tag=f"whf{r}")
                    nc.vector.tensor_tensor(whf[:], pw[:], b_bc1[:], op=ALU.add)
                    whm = const.tile([128, O + 1], mmdt, tag=f"whm{r}")
                    nc.vector.tensor_copy(whm[:], whf[:])
                    dlt = const.tile([128, O + 1], mmdt, tag=f"dlt{r}")
                    nc.vector.tensor_tensor(dlt[:], whf[:], whm[:],
                                            op=ALU.subtract)
                    wh_mm_tiles.append((whm, dlt))
                else:
                    whm = const.tile([128, O + 1], mmdt, tag=f"whm{r}")
                    nc.vector.tensor_tensor(whm[:], pw[:], b_bc1[:], op=ALU.add)
                    wh_mm_tiles.append(whm)

            return wh_mm_tiles

        def _body(_iv=None, wh_mm_tiles=None):
            if wh_mm_tiles is None:
                wh_mm_tiles = _stage1()
            # ---------- stage 2 ----------
            accs = []
            for c in range(CT) if not dma_only else range(0):
                acc_c = ps_acc.tile([O + 1, CW], F32, tag=f"acc{c}")
                accs.append(acc_c)
            adj_blk = adj.rearrange("(blk rb p) n -> blk p rb n", rb=RB, p=128)
            for blk in range(RT // RB):
                adj_t = adj_pool.tile([128, RB, N], I32)
                nc.sync.dma_start(adj_t[:], adj_blk[blk])
                if dma_only:
                    continue
                for rb in range(RB):
                    r = blk * RB + rb
                    p = p_pool.tile([128, N], mmdt, tag="p")
                    pl = None
                    if fused:
                        v = uv_pool.tile([128, N], F32, tag="v")
                        nc.scalar.activation(v[:], s2_bc[:], AF.Exp,
                                             bias=s1f_sb[:, r:r + 1],
                                             scale=ALPHA)
                        nc.vector._custom_dve(GAT_OP, out=p[:], in0=v[:],
                                              in1=adj_t[:, rb, :])
                        if p_comp:
                            pl = p_pool.tile([128, N], mmdt, tag="pl")
                            nc.vector._custom_dve(GAT_OP2, out=pl[:],
                                                  in0=v[:], in1=p[:])
                    else:
                        u = uv_pool.tile([128, N], F32, tag="u")
                        nc.scalar.activation(u[:], s2_bc[:], AF.Exp,
                                             bias=s1_sb[:, r:r + 1], scale=1.0)
                        v = uv_pool.tile([128, N], F32, tag="v")
                        nc.scalar.activation(v[:], s2_bc[:], AF.Exp,
                                             bias=s1f_sb[:, r:r + 1],
                                             scale=ALPHA)
                        pm = uv_pool.tile([128, N], F32, tag="pm")
                        nc.vector.tensor_tensor(pm[:], u[:], v[:], op=ALU.max)
                        nc.vector.tensor_tensor(p[:], pm[:], adj_t[:, rb, :],
                                                op=ALU.mult)
                    for c in range(CT):
                        csl = slice(c * CW, (c + 1) * CW)
                        if wh_comp:
                            whm_r, dlt_r = wh_mm_tiles[r]
                            nc.tensor.matmul(accs[c][:], whm_r[:], p[:, csl],
                                             start=(r == 0), stop=False)
                            if pl is not None:
                                nc.tensor.matmul(accs[c][:], whm_r[:],
                                                 pl[:, csl], start=False,
                                                 stop=False)
                            nc.tensor.matmul(accs[c][:], dlt_r[:], p[:, csl],
                                             start=False,
                                             stop=(r == RT - 1))
                        else:
                            nc.tensor.matmul(accs[c][:], wh_mm_tiles[r][:],
                                             p[:, csl], start=(r == 0),
                                             stop=(r == RT - 1))
            if dma_only:
                nc.vector.memset(out_sb[:], 0.0)
                nc.sync.dma_start(y.rearrange("(j p) o -> p j o", p=128),
                                  out_sb[:])
                return
            for c in range(CT):
                nc.scalar.copy(accT[:, c * CW:(c + 1) * CW], accs[c][:])

            # ---------- stage 3: transpose, normalize, ELU, store ----------
            for j in range(ET):
                jsl = slice(j * 128, (j + 1) * 128)
                pt = ps_ep.tile([128, O + 1], F32)
                nc.tensor.transpose(pt[:], accT[:, jsl], ident[:O + 1, :O + 1])
                rec = ep_pool.tile([128, 1], F32, tag="rec")
                nc.vector.reciprocal(rec[:], pt[:, O:O + 1])
                hp = ep_pool.tile([128, O], F32, tag="hp")
                nc.vector.tensor_scalar_mul(hp[:], pt[:, :O], rec[:])
                mn = ep_pool.tile([128, O], F32, tag="mn")
                nc.vector.tensor_scalar_min(mn[:], hp[:], 0.0)
                g = ep_pool.tile([128, O], F32, tag="g")
                nc.scalar.activation(g[:], mn[:], AF.Exp)
                nc.vector.scalar_tensor_tensor(out_sb[:, j, :], g[:], -1.0,
                                               hp[:], op0=ALU.add, op1=ALU.max)
            nc.sync.dma_start(y.rearrange("(j p) o -> p j o", p=128),
                              out_sb[:])

        if stage1_once:
            shared = _stage1()
            body = lambda iv=None: _body(iv, wh_mm_tiles=shared)
        else:
            body = _body
        if hw_loop and repeat > 1:
            tc.For_i_unrolled(0, repeat, 1, body, max_unroll=8)
        else:
            for _it in range(repeat):
                body()

    nc.compile()
    return nc


_NC_CACHE = None


def kernel(h, adj, W, b, a):
    global _NC_CACHE
    h = np.ascontiguousarray(h, dtype=np.float32)
    adj = np.ascontiguousarray(adj, dtype=np.int32)
    W = np.ascontiguousarray(W, dtype=np.float32)
    b = np.ascontiguousarray(b, dtype=np.float32)
    a = np.ascontiguousarray(a, dtype=np.float32)

    if _NC_CACHE is None:
        _NC_CACHE = build_gat_kernel(fused=False, mm_f32r=False,
                                     adj_bufs=3, uv_bufs=2)
    nc = _NC_CACHE

    in_maps = [
        {"h": h[i], "adj": adj[i], "W": W, "b": b, "a": a} for i in range(B)
    ]
    res = run_bass_kernel_spmd(nc, in_maps, core_ids=list(range(B)))
    out = np.stack([r["y"] for r in res.results], axis=0)
    return out

